# revision 1
# baseline (speedup 1.0000x reference)
"""Single-head causal attention kernel for Trainium2, 8-core data-parallel.

Problem: x[8, 2048, 1024], w_q/w_k/w_v[64, 1024] (torch Linear convention)
  q = x @ w_q.T; k = x @ w_k.T; v = x @ w_v.T          [B, S, H]
  out = softmax(mask(q @ k.T / sqrt(H))) @ v           [B, S, H]

Sharding: data-parallel over batch, one batch element per NeuronCore.

Per-core plan (S=2048, E=1024, H=64):
  - x loaded with fp32->bf16 cast during DMA (SWDGE), 16 tiles [128, 1024].
  - xT (partition=E layout needed for projections) built with PE transposes
    (bf16, 128x [128,128] blocks), evicted PSUM->SBUF by ACT/DVE.
  - Projections as two packed M=128 passes:
      pass1 -> [qT; kT]  (rows 0-63 = qT, 64-127 = kT)   [128, 2048]
      pass2 -> [kT; vT]  (rows 0-63 = kT, 64-127 = vT)   [128, 2048]
    so the score matmul has both operands on partitions 0-63.
  - v_aug[t] = [v | 1] tiles [128, 65] via PE transpose of vT; the ones
    column makes the AV matmul produce the softmax denominator for free.
  - scoresT[j, i] blocks (transposed scores) = kT_slice.T @ qT  (K=64, N=512)
  - exp fused into the PSUM->SBUF eviction on ScalarE (scale=1/8 folded in),
    no max-subtraction (scores ~ N(0,1), exp is safe in fp32).
  - causal masking of diagonal blocks with gpsimd affine_select (fill 0).
  - AV: oT[h|den, i] = sum_t v_aug[t].T @ attnT[t]  (fp32 PSUM accumulate)
  - oT transposed back with PE (fp32, exact), normalized by DVE
    tensor_scalar_mul with per-partition reciprocal of the denominator.
"""

import numpy as np

import concourse.bass as bass
import concourse.bacc as bacc_mod
import concourse.tile as tile
from concourse import mybir
from concourse.bass import ts
from concourse.bass_utils import run_bass_kernel_spmd
from concourse.masks import make_identity

B, S, E, H = 8, 2048, 1024, 64
P = 128
NB = S // 512          # 4 column blocks of 512
NT = S // P            # 16 row tiles of 128
ET = E // P            # 8 contraction tiles of 128
FP32 = mybir.dt.float32
BF16 = mybir.dt.bfloat16

N_CORES = 8


def _emit(nc, tc, ctx, x_d, wq_d, wk_d, wv_d, out_d):
    consts = ctx.enter_context(tc.tile_pool(name="consts", bufs=1))
    wnat = ctx.enter_context(tc.tile_pool(name="wnat", bufs=3))
    wt = ctx.enter_context(tc.tile_pool(name="wt", bufs=1))
    xpool = ctx.enter_context(tc.tile_pool(name="xp", bufs=16))
    xt = ctx.enter_context(tc.tile_pool(name="xt", bufs=1))
    vaug = ctx.enter_context(tc.tile_pool(name="vaug", bufs=NT))
    att = ctx.enter_context(tc.tile_pool(name="att", bufs=32))
    fin = ctx.enter_context(tc.tile_pool(name="fin", bufs=8))
    outp = ctx.enter_context(tc.tile_pool(name="outp", bufs=8))

    tp_ps = ctx.enter_context(tc.tile_pool(name="tp_ps", bufs=2, space="PSUM"))
    proj_ps = ctx.enter_context(tc.tile_pool(name="proj_ps", bufs=2, space="PSUM"))
    sc_ps = ctx.enter_context(tc.tile_pool(name="sc_ps", bufs=2, space="PSUM"))
    av_ps = ctx.enter_context(tc.tile_pool(name="av_ps", bufs=2, space="PSUM"))

    # --- constants -------------------------------------------------------
    ident_bf = consts.tile([P, P], BF16)
    make_identity(nc, ident_bf)
    ident_f32 = consts.tile([P, P], FP32)
    make_identity(nc, ident_f32)
    # shifted identity: ident2[64+r, r] = 1, for transposing tiles that live
    # on partitions 64-127 (PE requires lhsT/rhs on the same partitions)
    ident2_bf = consts.tile([P, H], BF16)
    nc.gpsimd.memset(ident2_bf, 0.0)
    nc.gpsimd.affine_select(
        out=ident2_bf,
        in_=ident2_bf,
        compare_op=mybir.AluOpType.not_equal,
        fill=1.0,
        base=-H,
        pattern=[[-1, H]],
        channel_multiplier=1,
    )

    # --- weights: load with cast, transpose on PE ------------------------
    # wq/wk/wv natural [64, 1024] bf16
    w_nat = []
    for w_d in (wq_d, wk_d, wv_d):
        wn = wnat.tile([H, E], BF16, tag="wnat")
        nc.gpsimd.dma_start(out=wn, in_=w_d)  # fp32 -> bf16 cast in DMA
        w_nat.append(wn)
    wq_n, wk_n, wv_n = w_nat

    # wqk[et]: [128, 128] cols 0-63 = wq.T slice, cols 64-127 = wk.T slice
    # wkv[et]: [128, 128] cols 0-63 = wk.T slice, cols 64-127 = wv.T slice
    wqk = [wt.tile([P, P], BF16, tag=f"wqk{et}", name=f"wqk{et}") for et in range(ET)]
    wkv = [wt.tile([P, P], BF16, tag=f"wkv{et}", name=f"wkv{et}") for et in range(ET)]
    for et in range(ET):
        for wn, dest, cols in (
            (wq_n, wqk[et], slice(0, H)),
            (wk_n, wqk[et], slice(H, P)),
            (wk_n, wkv[et], slice(0, H)),
            (wv_n, wkv[et], slice(H, P)),
        ):
            ps = tp_ps.tile([P, 512], BF16, tag="tp")
            nc.tensor.transpose(ps[:, 0:H], wn[:, ts(et, P)], ident_bf[0:H, 0:H])
            nc.scalar.copy(dest[:, cols], ps[:, 0:H])

    # --- x: load with cast, PE-transpose to xT ---------------------------
    x_tiles = []
    for t in range(NT):
        xtile = xpool.tile([P, E], BF16, tag="x")
        nc.gpsimd.dma_start(out=xtile, in_=x_d[ts(t, P), :])  # cast fp32->bf16
        x_tiles.append(xtile)

    # xT_all column layout: col = et*2048 + t*128 + s  (et-major), i.e.
    # xT_all[:, et*2048 + nb*512 : et*2048 + (nb+1)*512] is the [128, 512]
    # block x[512*nb:512*(nb+1), 128*et:128*(et+1)].T used by the projections.
    xT_all = xt.tile([P, ET * S], BF16, tag="xT_all")
    xT_view = xT_all.rearrange("p (e t s) -> p e t s", e=ET, t=NT)

    def emit_x_transpose(t):
        for eg in range(2):  # groups of 4 e-tiles per PSUM tile
            ps = tp_ps.tile([P, 512], BF16, tag="tp", name=f"tp{t}_{eg}")
            for j in range(4):
                et = eg * 4 + j
                nc.tensor.transpose(
                    ps[:, ts(j, P)], x_tiles[t][:, ts(et, P)], ident_bf
                )
            # single eviction: 4 transposed blocks scatter to 4 et-slices
            srcv = ps.rearrange("p (e s) -> p e s", e=4)
            dst = xT_view[:, eg * 4:(eg + 1) * 4, t, :]
            nc.vector.tensor_copy(dst, srcv)

    # --- projections -----------------------------------------------------
    # qk1 rows 0-63 = qT, rows 64-127 = kT ; kv2 rows 0-63 = kT, 64-127 = vT
    qk1 = consts.tile([P, S], BF16, tag="qk1")
    kv2 = consts.tile([P, S], BF16, tag="kv2")
    qhi = consts.tile([P, S], BF16, tag="qhi")
    v_aug = [vaug.tile([P, H + 1], BF16, tag="vaug", name=f"vaug{_t}") for _t in range(NT)]

    for nb in range(NB):
        for t in range(4 * nb, 4 * nb + 4):
            emit_x_transpose(t)
        for wts, dest in ((wqk, qk1), (wkv, kv2)):
            ps = proj_ps.tile([P, 512], FP32, tag="proj")
            for et in range(ET):
                rhs = xT_all[:, et * S + nb * 512: et * S + (nb + 1) * 512]
                nc.tensor.matmul(
                    ps, wts[et], rhs,
                    start=(et == 0), stop=(et == ET - 1),
                )
            nc.vector.tensor_copy(dest[:, ts(nb, 512)], ps)
        # qT duplicated onto partitions 64-127 for row-tiled score matmuls
        nc.vector.tensor_copy(qhi[H:P, ts(nb, 512)], qk1[0:H, ts(nb, 512)])
        # v_aug tiles for this column block: transpose vT (rows 64-127 of kv2)
        for j in range(4):
            t = nb * 4 + j
            ps = proj_ps.tile([P, 512], BF16, tag="proj")
            nc.tensor.transpose(
                ps[:, 0:H], kv2[H:P, ts(t, P)], ident2_bf[H:P, :]
            )
            nc.vector.tensor_copy(v_aug[t][:, 0:H], ps[:, 0:H])
            nc.vector.memset(v_aug[t][:, H:H + 1], 1.0)

        # --- attention for query column block b = nb -------------------
        b = nb
        n_jt = 4 * b + 4  # causal: j-tiles 0 .. 4b+3
        at_tiles = []
        for t in range(n_jt):
            ps = sc_ps.tile([P, 512], FP32, tag="sc")
            c0 = P * (t - 4 * b) if t >= 4 * b else 0  # cols < c0 all-masked
            # scoresT[j, i] = sum_h kT[h, j] * qT[h, i]; odd t runs on array
            # rows 64-127 (kT from pass1 rows 64-127, qT dup in qhi) so two
            # K=64 matmuls occupy both halves of the PE array concurrently.
            # Only the causally-reachable columns [c0, 512) are computed.
            if t % 2 == 0:
                nc.tensor.matmul(
                    ps[:, c0:], kv2[0:H, ts(t, P)],
                    qk1[0:H, 512 * b + c0: 512 * (b + 1)],
                    start=True, stop=True,
                )
            else:
                nc.tensor.matmul(
                    ps[:, c0:], qk1[H:P, ts(t, P)],
                    qhi[H:P, 512 * b + c0: 512 * (b + 1)],
                    start=True, stop=True,
                )
            at = att.tile([P, 512], BF16, tag="attnT")
            nc.scalar.activation(
                at[:, c0:], ps[:, c0:], mybir.ActivationFunctionType.Exp,
                scale=0.125,
            )
            if t >= 4 * b:  # diagonal block: zero where j > i. Only the
                # 128-col triangle strip [c0, c0+128) needs masking; cols
                # below c0 are never read by the narrowed AV matmul.
                nc.gpsimd.affine_select(
                    out=at[:, c0:c0 + P],
                    in_=at[:, c0:c0 + P],
                    compare_op=mybir.AluOpType.is_ge,
                    fill=0.0,
                    base=0,
                    pattern=[[1, P]],
                    channel_multiplier=-1,
                )
            at_tiles.append(at)

        # oT[m, i] for m in 0..63 = sum_j attn * v ; m = 64: denominator
        oT = av_ps.tile([H + 1, 512], FP32, tag="av")
        for t in range(n_jt):
            c0 = P * (t - 4 * b) if t >= 4 * b else 0
            nc.tensor.matmul(
                oT[:, c0:], v_aug[t], at_tiles[t][:, c0:],
                start=(t == 0), stop=(t == n_jt - 1),
            )
        oT_sb = fin.tile([H + 1, 512], FP32, tag="oT")
        nc.vector.tensor_copy(oT_sb, oT)

        for j in range(4):
            t = b * 4 + j
            ps = av_ps.tile([P, H + 1], FP32, tag="av")
            nc.tensor.transpose(
                ps, oT_sb[:, ts(j, P)], ident_f32[0:H + 1, 0:H + 1]
            )
            r = fin.tile([P, 1], FP32, tag="recip")
            nc.vector.reciprocal(r, ps[:, H:H + 1])
            o_sb = outp.tile([P, H], FP32, tag="out")
            nc.vector.tensor_scalar_mul(o_sb, ps[:, 0:H], r)
            nc.sync.dma_start(out=out_d[ts(t, P), :], in_=o_sb)


_NC_CACHE = {}


def _build_nc():
    if "nc" not in _NC_CACHE:
        from contextlib import ExitStack

        nc = bacc_mod.Bacc("TRN2")
        x_d = nc.dram_tensor("x", [S, E], FP32, kind="ExternalInput")
        wq_d = nc.dram_tensor("w_q", [H, E], FP32, kind="ExternalInput")
        wk_d = nc.dram_tensor("w_k", [H, E], FP32, kind="ExternalInput")
        wv_d = nc.dram_tensor("w_v", [H, E], FP32, kind="ExternalInput")
        out_d = nc.dram_tensor("out", [S, H], FP32, kind="ExternalOutput")
        with tile.TileContext(nc) as tc:
            with ExitStack() as ctx:
                _emit(nc, tc, ctx, x_d[:, :], wq_d[:, :], wk_d[:, :],
                      wv_d[:, :], out_d[:, :])
        nc.compile()
        _NC_CACHE["nc"] = nc
    return _NC_CACHE["nc"]


def kernel(x, w_q, w_k, w_v, _trace=False, _trace_kwargs=None):
    nc = _build_nc()
    x = np.ascontiguousarray(x, dtype=np.float32)
    in_maps = [
        {
            "x": x[b],
            "w_q": np.ascontiguousarray(w_q, dtype=np.float32),
            "w_k": np.ascontiguousarray(w_k, dtype=np.float32),
            "w_v": np.ascontiguousarray(w_v, dtype=np.float32),
        }
        for b in range(N_CORES)
    ]
    res = run_bass_kernel_spmd(
        nc, in_maps, list(range(N_CORES)), trace=_trace,
        **(_trace_kwargs or {}),
    )
    out = np.stack([res.results[b]["out"] for b in range(N_CORES)])
    if _trace:
        return out.astype(np.float32), res
    return out.astype(np.float32)

